# revision 9
# baseline (speedup 1.0000x reference)
"""Trainium2 Bass kernel for nn_ConceptEmbedding (type-conditioned embedding lookup).

Reference computation (per token position (b, s)):
    t = token_type[b, s]
    out[b, s, :] = proc_emb[concept]  if t == 1
                   med_emb[concept]   if t == 2
                   chart_emb[concept] if t == 3
                   0                  otherwise

Strategy (v3):
  - Fold the three tables into one [3V, E] table with flat row index
    (t-1)*V + concept. Tokens with t outside {1,2,3} produce zeros and are
    never sent to the device (the host assembles their rows as zeros).
  - Shard tokens across the 8 cores BY TABLE ROW RANGE: core c owns rows
    [c*37504, (c+1)*37504). The host hands each core a contiguous slice of
    the table ("twin", 37632 rows) as its per-core input, so all gather
    windows have static bases. ~3072 typed tokens land on each core.
  - Device (per core): the HW gather (InstDMAGatherAnt) takes int16 indices,
    so the 37632-row band is covered by two windows (0:32768 and
    32768:37632). Four dma_gather instructions (3x1024 + 1x768 slots) pull
    the rows into one SBUF buffer; one contiguous DMA stores it to DRAM.
    Unused slots are padded with index 0 (a benign in-band row) and their
    output is simply discarded by the host.
  - Host: buckets/sorts tokens by row (ascending - also gives the DMA
    ascending HBM addresses), pads buckets to the fixed caps, unpermutes the
    result while assembling the full [B, S, E] output.

dma_gather layout facts (verified on HW):
  - indices live at idxs[i % 16, i // 16], int16, replicated across all 128
    partitions; valid index i lands at dst[i % 128, i // 128, :].
  - one instruction must stay near ~1024 indices (the SWDGE descriptor ring
    is small; 1792-index gathers crash the exec unit).
  - the store view maps SBUF (p, block b) -> DRAM row p*NB + b, so the DRAM
    row for slot j of a window at block base B0 is (j % 128)*NB + B0 + j//128.
"""

import numpy as np

V = 100000
E = 128
B = 16
S = 2048
NCORES = 8
P = 128

N_TOK = B * S  # 32768
NROWS = 3 * V  # 300000

RSPAN = 37504  # table rows owned per core (8 * 37504 >= 300000)
TWLEN = 37632  # per-core table slice length (RSPAN + 128 alignment margin)
W0 = 32768  # window 0 covers twin[0:32768]
W1LEN = TWLEN - W0  # 4864 rows in window 1

# Gather instructions per core: slots per instruction (multiples of 128).
# Window 0 expected ~2685 typed tokens (cap 3072), window 1 expected ~390
# (cap 768); caps sit >7 sigma above the means for the uniform input law.
W0CAP = 3072
W1CAP = 768
GATHERS = [(0, 1024), (0, 1024), (0, 1024), (1, 768)]
SUMCAP = W0CAP + W1CAP  # 3840
NB = SUMCAP // P  # 30 blocks
W0BLOCKS = W0CAP // P  # 24

_CACHED_NC = None


def _build_bass():
    global _CACHED_NC
    if _CACHED_NC is not None:
        return _CACHED_NC

    import concourse.bacc as bacc
    import concourse.mybir as mybir
    import concourse.tile as tile

    # Bacc (not raw Bass): its finalize() runs generate_event_semaphores,
    # which splits multi-wait sync_infos down to the 1-wait-per-instruction
    # HW limit that walrus codegen enforces.
    nc = bacc.Bacc()
    twin = nc.dram_tensor("twin", [TWLEN, E], mybir.dt.float32, kind="ExternalInput")
    idx = nc.dram_tensor("idx", [P, SUMCAP // 16], mybir.dt.int16, kind="ExternalInput")
    out = nc.dram_tensor("out", [SUMCAP, E], mybir.dt.float32, kind="ExternalOutput")

    # SBUF (p, block b) <-> DRAM row p*NB + b
    out_v = out.rearrange("(p b) e -> p (b e)", p=P)

    with tile.TileContext(nc) as tc:
        with (
            tc.tile_pool(name="idxp", bufs=1) as idxp,
            tc.tile_pool(name="rows", bufs=1) as rows,
        ):
            idx_tile = idxp.tile([P, SUMCAP // 16], mybir.dt.int16)
            nc.sync.dma_start(out=idx_tile[:], in_=idx[:])
            big = rows.tile([P, NB * E], mybir.dt.float32)
            off16 = 0
            b0 = 0
            for w, cap in GATHERS:
                bw = cap // P
                in_ap = twin[0:W0, :] if w == 0 else twin[W0:TWLEN, :]
                dst = big[:, b0 * E : (b0 + bw) * E].rearrange("p (b e) -> p b e", e=E)
                nc.gpsimd.dma_gather(
                    out_ap=dst,
                    in_ap=in_ap,
                    idxs_ap=idx_tile[:, off16 : off16 + cap // 16],
                    num_idxs=cap,
                    num_idxs_reg=cap,
                    elem_size=E,
                )
                off16 += cap // 16
                b0 += bw
            nc.sync.dma_start(out=out_v[:], in_=big[:])

    nc.finalize()
    _CACHED_NC = nc
    return nc


def _shard_inputs(proc_emb, med_emb, chart_emb, concept, token_type):
    """Returns (in_maps, plans, tables) with per-core slot bookkeeping."""
    tables = np.ascontiguousarray(
        np.concatenate(
            [
                np.asarray(proc_emb, dtype=np.float32),
                np.asarray(med_emb, dtype=np.float32),
                np.asarray(chart_emb, dtype=np.float32),
            ],
            axis=0,
        )
    )
    tt = np.asarray(token_type).reshape(-1).astype(np.int64)
    cc = np.asarray(concept).reshape(-1).astype(np.int64)
    typed = (tt >= 1) & (tt <= 3)
    toks_all = np.where(typed)[0]  # global token ids with a real lookup
    eff = cc[toks_all] + (tt[toks_all] - 1) * V  # their table rows

    core_of = eff // RSPAN
    local = eff - core_of * RSPAN

    in_maps = []
    plans = []  # per core: (tokens, dram_rows, overflow_tokens, overflow_rows)
    for c in range(NCORES):
        base = c * RSPAN
        sl = tables[base : min(base + TWLEN, NROWS)]
        if sl.shape[0] < TWLEN:
            sl = np.concatenate([sl, np.zeros((TWLEN - sl.shape[0], E), np.float32)])
        twin = np.ascontiguousarray(sl)

        sel = np.where(core_of == c)[0]
        order = sel[np.argsort(local[sel], kind="stable")]
        lrows = local[order]  # ascending
        n0 = int(np.searchsorted(lrows, W0))  # tokens in window 0
        win_lists = [
            (lrows[:n0], toks_all[order[:n0]], W0CAP, 0, 0),
            (lrows[n0:] - W0, toks_all[order[n0:]], W1CAP, W0CAP, W0BLOCKS),
        ]

        idx16 = np.zeros((16, SUMCAP // 16), dtype=np.int16)
        tok_list, row_list, ovf_toks, ovf_rows = [], [], [], []
        for wrows, wtoks, cap, slot0, b0 in win_lists:
            cnt = len(wrows)
            if cnt > cap:
                # Statistical-tail safety valve: gather the overflow on host.
                ovf_toks.extend(wtoks[cap:].tolist())
                ovf_rows.extend((wrows[cap:] + (0 if slot0 == 0 else W0)).tolist())
                wrows, wtoks, cnt = wrows[:cap], wtoks[:cap], cap
            vals = np.zeros(cap, dtype=np.int16)
            vals[:cnt] = wrows.astype(np.int16)  # pad keeps 0 (benign row)
            idx16[:, slot0 // 16 : (slot0 + cap) // 16] = vals.reshape(cap // 16, 16).T
            j = np.arange(cnt)
            row_list.append((j % P) * NB + b0 + j // P)
            tok_list.append(wtoks)

        in_maps.append(
            {"twin": twin, "idx": np.ascontiguousarray(np.tile(idx16, (8, 1)))}
        )
        plans.append(
            (
                np.concatenate(tok_list),
                np.concatenate(row_list),
                np.array(ovf_toks, dtype=np.int64),
                np.array(ovf_rows, dtype=np.int64) + base,
            )
        )

    return in_maps, plans, tables


def _run(in_maps, trace=False):
    from concourse.bass_utils import run_bass_kernel_spmd

    nc = _build_bass()
    return run_bass_kernel_spmd(nc, in_maps, list(range(NCORES)), trace=trace)


def _assemble(results, plans, tables):
    out = np.zeros((N_TOK, E), dtype=np.float32)
    for c in range(NCORES):
        toks, drows, ovf_toks, ovf_rows = plans[c]
        if len(toks):
            out[toks] = results[c]["out"][drows]
        if len(ovf_toks):
            out[ovf_toks] = tables[ovf_rows]
    return out.reshape(B, S, E)


def kernel(proc_emb, med_emb, chart_emb, concept, token_type):
    in_maps, plans, tables = _shard_inputs(
        proc_emb, med_emb, chart_emb, concept, token_type
    )
    res = _run(in_maps, trace=False)
    return _assemble(res.results, plans, tables)


# revision 12
# speedup vs baseline: 1.3919x; 1.3919x over previous
"""Trainium2 Bass kernel for nn_ConceptEmbedding (type-conditioned embedding lookup).

Reference computation (per token position (b, s)):
    t = token_type[b, s]
    out[b, s, :] = proc_emb[concept]  if t == 1
                   med_emb[concept]   if t == 2
                   chart_emb[concept] if t == 3
                   0                  otherwise

Strategy (v3):
  - Fold the three tables into one [3V, E] table with flat row index
    (t-1)*V + concept. Tokens with t outside {1,2,3} produce zeros and are
    never sent to the device (the host assembles their rows as zeros).
  - Shard tokens across the 8 cores BY TABLE ROW RANGE: core c owns rows
    [c*37504, (c+1)*37504). The host hands each core a contiguous slice of
    the table ("twin", 37632 rows) as its per-core input, so all gather
    windows have static bases. ~3072 typed tokens land on each core.
  - Device (per core): the HW gather (InstDMAGatherAnt) takes int16 indices,
    so the 37632-row band is covered by two windows (0:32768 and
    32768:37632). Four dma_gather instructions (3x1024 + 1x768 slots) pull
    the rows into one SBUF buffer; one contiguous DMA stores it to DRAM.
    Unused slots are padded with index 0 (a benign in-band row) and their
    output is simply discarded by the host.
  - Host: buckets/sorts tokens by row (ascending - also gives the DMA
    ascending HBM addresses), pads buckets to the fixed caps, unpermutes the
    result while assembling the full [B, S, E] output.

dma_gather layout facts (verified on HW):
  - indices live at idxs[i % 16, i // 16], int16, replicated across all 128
    partitions; valid index i lands at dst[i % 128, i // 128, :].
  - one instruction must stay near ~1024 indices (the SWDGE descriptor ring
    is small; 1792-index gathers crash the exec unit).
  - the store view maps SBUF (p, block b) -> DRAM row p*NB + b, so the DRAM
    row for slot j of a window at block base B0 is (j % 128)*NB + B0 + j//128.
"""

import numpy as np

V = 100000
E = 128
B = 16
S = 2048
NCORES = 8
P = 128

N_TOK = B * S  # 32768
NROWS = 3 * V  # 300000

RSPAN = 37504  # table rows owned per core (8 * 37504 >= 300000)
TWLEN = 37632  # per-core table slice length (RSPAN + 128 alignment margin)
W0 = 32768  # window 0 covers twin[0:32768]
W1LEN = TWLEN - W0  # 4864 rows in window 1

# Gather instructions per core: slots per instruction (multiples of 128).
# Window 0 expected ~2685 typed tokens (cap 3072), window 1 expected ~390
# (cap 768); caps sit >7 sigma above the means for the uniform input law.
W0CAP = 3072
W1CAP = 768
GATHERS = [(0, 1024), (0, 1024), (0, 1024), (1, 768)]
SUMCAP = W0CAP + W1CAP  # 3840
NB = SUMCAP // P  # 30 blocks
W0BLOCKS = W0CAP // P  # 24

_CACHED_NC = None


def _build_bass():
    global _CACHED_NC
    if _CACHED_NC is not None:
        return _CACHED_NC

    import concourse.bacc as bacc
    import concourse.mybir as mybir
    import concourse.tile as tile

    # Bacc (not raw Bass): its finalize() runs generate_event_semaphores,
    # which splits multi-wait sync_infos down to the 1-wait-per-instruction
    # HW limit that walrus codegen enforces.
    # 4 SWDGE queues: spread the four gathers across queue contexts so Q7
    # descriptor generation (the measured bottleneck, ~8 ns/index) can
    # overlap across instructions.
    nc = bacc.Bacc(num_swdge_queues=4)
    twin = nc.dram_tensor("twin", [TWLEN, E], mybir.dt.float32, kind="ExternalInput")
    idx = nc.dram_tensor("idx", [P, SUMCAP // 16], mybir.dt.int16, kind="ExternalInput")
    out = nc.dram_tensor("out", [SUMCAP, E], mybir.dt.float32, kind="ExternalOutput")

    # SBUF (p, block b) <-> DRAM row p*NB + b
    out_v = out.rearrange("(p b) e -> p (b e)", p=P)

    with tile.TileContext(nc) as tc:
        with (
            tc.tile_pool(name="idxp", bufs=1) as idxp,
            tc.tile_pool(name="rows", bufs=1) as rows,
        ):
            idx_tile = idxp.tile([P, SUMCAP // 16], mybir.dt.int16)
            nc.sync.dma_start(out=idx_tile[:], in_=idx[:])
            big = rows.tile([P, NB * E], mybir.dt.float32)
            off16 = 0
            b0 = 0
            for qn, (w, cap) in enumerate(GATHERS):
                bw = cap // P
                in_ap = twin[0:W0, :] if w == 0 else twin[W0:TWLEN, :]
                dst = big[:, b0 * E : (b0 + bw) * E].rearrange("p (b e) -> p b e", e=E)
                nc.gpsimd.dma_gather(
                    out_ap=dst,
                    in_ap=in_ap,
                    idxs_ap=idx_tile[:, off16 : off16 + cap // 16],
                    num_idxs=cap,
                    num_idxs_reg=cap,
                    elem_size=E,
                    queue_num=qn % 4,
                )
                off16 += cap // 16
                b0 += bw
            nc.sync.dma_start(out=out_v[:], in_=big[:])

    nc.finalize()
    _CACHED_NC = nc
    return nc


def _shard_inputs(proc_emb, med_emb, chart_emb, concept, token_type):
    """Returns (in_maps, plans, tables) with per-core slot bookkeeping."""
    tables = np.ascontiguousarray(
        np.concatenate(
            [
                np.asarray(proc_emb, dtype=np.float32),
                np.asarray(med_emb, dtype=np.float32),
                np.asarray(chart_emb, dtype=np.float32),
            ],
            axis=0,
        )
    )
    tt = np.asarray(token_type).reshape(-1).astype(np.int64)
    cc = np.asarray(concept).reshape(-1).astype(np.int64)
    typed = (tt >= 1) & (tt <= 3)
    toks_all = np.where(typed)[0]  # global token ids with a real lookup
    eff = cc[toks_all] + (tt[toks_all] - 1) * V  # their table rows

    core_of = eff // RSPAN
    local = eff - core_of * RSPAN

    in_maps = []
    plans = []  # per core: (tokens, dram_rows, overflow_tokens, overflow_rows)
    for c in range(NCORES):
        base = c * RSPAN
        sl = tables[base : min(base + TWLEN, NROWS)]
        if sl.shape[0] < TWLEN:
            sl = np.concatenate([sl, np.zeros((TWLEN - sl.shape[0], E), np.float32)])
        twin = np.ascontiguousarray(sl)

        sel = np.where(core_of == c)[0]
        order = sel[np.argsort(local[sel], kind="stable")]
        lrows = local[order]  # ascending
        n0 = int(np.searchsorted(lrows, W0))  # tokens in window 0
        win_lists = [
            (lrows[:n0], toks_all[order[:n0]], W0CAP, 0, 0),
            (lrows[n0:] - W0, toks_all[order[n0:]], W1CAP, W0CAP, W0BLOCKS),
        ]

        idx16 = np.zeros((16, SUMCAP // 16), dtype=np.int16)
        tok_list, row_list, ovf_toks, ovf_rows = [], [], [], []
        for wrows, wtoks, cap, slot0, b0 in win_lists:
            cnt = len(wrows)
            if cnt > cap:
                # Statistical-tail safety valve: gather the overflow on host.
                ovf_toks.extend(wtoks[cap:].tolist())
                ovf_rows.extend((wrows[cap:] + (0 if slot0 == 0 else W0)).tolist())
                wrows, wtoks, cnt = wrows[:cap], wtoks[:cap], cap
            vals = np.zeros(cap, dtype=np.int16)
            vals[:cnt] = wrows.astype(np.int16)  # pad keeps 0 (benign row)
            idx16[:, slot0 // 16 : (slot0 + cap) // 16] = vals.reshape(cap // 16, 16).T
            j = np.arange(cnt)
            row_list.append((j % P) * NB + b0 + j // P)
            tok_list.append(wtoks)

        in_maps.append(
            {"twin": twin, "idx": np.ascontiguousarray(np.tile(idx16, (8, 1)))}
        )
        plans.append(
            (
                np.concatenate(tok_list),
                np.concatenate(row_list),
                np.array(ovf_toks, dtype=np.int64),
                np.array(ovf_rows, dtype=np.int64) + base,
            )
        )

    return in_maps, plans, tables


def _run(in_maps, trace=False):
    from concourse.bass_utils import run_bass_kernel_spmd

    nc = _build_bass()
    return run_bass_kernel_spmd(nc, in_maps, list(range(NCORES)), trace=trace)


def _assemble(results, plans, tables):
    out = np.zeros((N_TOK, E), dtype=np.float32)
    for c in range(NCORES):
        toks, drows, ovf_toks, ovf_rows = plans[c]
        if len(toks):
            out[toks] = results[c]["out"][drows]
        if len(ovf_toks):
            out[ovf_toks] = tables[ovf_rows]
    return out.reshape(B, S, E)


def kernel(proc_emb, med_emb, chart_emb, concept, token_type):
    in_maps, plans, tables = _shard_inputs(
        proc_emb, med_emb, chart_emb, concept, token_type
    )
    res = _run(in_maps, trace=False)
    return _assemble(res.results, plans, tables)
